# revision 6
# baseline (speedup 1.0000x reference)
"""BERT-base + CRF Viterbi decode on 8 Trainium2 NeuronCores.

Sharding: data-parallel over batch (16 seqs -> 2 per core). Each core runs
the full 12-layer encoder on its 512 tokens (feature-major activations,
fp32 matmuls for Viterbi argmax exactness) plus the CRF Viterbi DP on DVE.

Self-contained: hardcodes all shapes from the problem spec.
"""
import os
from contextlib import ExitStack

import numpy as np

import concourse.bass as bass
import concourse.mybir as mybir
import concourse.tile as tile
from concourse import bacc
from concourse.bass_utils import run_bass_kernel_spmd

F32 = mybir.dt.float32
I32 = mybir.dt.int32
AF = mybir.ActivationFunctionType
OP = mybir.AluOpType
AX = mybir.AxisListType

V, TMAX, H, L_FULL, FF, NHEAD, DH = 30522, 512, 768, 12, 3072, 12, 64
NL, START_ID, STOP_ID = 11, 9, 10
B, T = 16, 256
NEG = -10000.0
NCORES = 8
SEQ_PER_CORE = B // NCORES          # 2
NTOK = SEQ_PER_CORE * T             # 512
KH = H // 128                       # 6 h-tiles
KF = FF // 128                      # 24 f-tiles
NTT = NTOK // 128                   # 4 token-tiles
EPS = 1e-12

N_LAYERS = int(os.environ.get("KERNEL_N_LAYERS", str(L_FULL)))


def _np(x):
    return np.ascontiguousarray(np.asarray(x))


# ---------------------------------------------------------------------------
# device program
# ---------------------------------------------------------------------------

def build_module(flags):
    nc = bacc.Bacc(None, target_bir_lowering=False, debug=False)

    d = {}
    def inp(name, shape, dt=F32):
        d[name] = nc.dram_tensor(name, shape, dt, kind="ExternalInput")
        return d[name]

    inp("ids", [NTOK, 1], I32)
    inp("word_emb", [V, H])
    inp("pt_emb", [NTOK, H])
    inp("w_qkvo", [N_LAYERS, 4, H, H])
    inp("w_ffn1", [N_LAYERS, KF, H, 128])
    inp("w_ffn2", [N_LAYERS, KH, FF, 128])
    inp("biases", [N_LAYERS, 6, FF])   # rows: bq bk bo b1 b2 bv
    inp("ln_gb", [N_LAYERS, 2, 2, H])
    inp("w_out", [H, NL])
    inp("b_out", [NL, 1])
    inp("emb_ln", [2, H])
    inp("mask_bias", [NTOK, 1])
    inp("trans2", [2, NL * NL])
    inp("rev121", [2, NL * NL])
    inp("rev11", [2, NL])
    inp("vit_init", [2, NL])
    inp("ident", [128, 128])
    inp("ones_col", [128, 1])
    inp("ones_row", [1, 128])

    d["score_out"] = nc.dram_tensor("score_out", [2, 1], F32, kind="ExternalOutput")
    d["path_out"] = nc.dram_tensor("path_out", [2, T], I32, kind="ExternalOutput")

    with tile.TileContext(nc) as tc:
        _emit(nc, tc, flags, d)
    nc.finalize()
    return nc


def _emit(nc, tc, flags, d):
    with ExitStack() as top:
        # long-lived pools
        const_p = top.enter_context(tc.tile_pool(name="const", bufs=1))
        act_p = top.enter_context(tc.tile_pool(name="act", bufs=1))
        qkv_p = top.enter_context(tc.tile_pool(name="qkv", bufs=1))
        exp_p = top.enter_context(tc.tile_pool(name="exp", bufs=4))
        misc_p = top.enter_context(tc.tile_pool(name="misc", bufs=2))
        stat_p = top.enter_context(tc.tile_pool(name="stat", bufs=2))
        sq_p = top.enter_context(tc.tile_pool(name="sq", bufs=2))
        ps_proj = top.enter_context(tc.tile_pool(name="ps_proj", bufs=3, space="PSUM"))
        ps_att = top.enter_context(tc.tile_pool(name="ps_att", bufs=3, space="PSUM"))
        ps_sm = top.enter_context(tc.tile_pool(name="ps_sm", bufs=1, space="PSUM"))

        # ---- constants ----
        ident = const_p.tile([128, 128], F32)
        nc.sync.dma_start(ident[:], d["ident"][:])
        ones_col = const_p.tile([128, 1], F32)
        nc.sync.dma_start(ones_col[:], d["ones_col"][:])
        ones_row = const_p.tile([1, 128], F32)
        nc.sync.dma_start(ones_row[:], d["ones_row"][:])
        trans2 = const_p.tile([2, NL * NL], F32)
        nc.sync.dma_start(trans2[:], d["trans2"][:])
        rev121 = const_p.tile([2, NL * NL], F32)
        nc.sync.dma_start(rev121[:], d["rev121"][:])
        rev11 = const_p.tile([2, NL], F32)
        nc.sync.dma_start(rev11[:], d["rev11"][:])
        eps_col = const_p.tile([128, 1], F32)
        nc.vector.memset(eps_col[:], EPS)
        wout = const_p.tile([128, KH, NL], F32)
        nc.sync.dma_start(wout[:], d["w_out"].rearrange("(ko p) m -> p ko m", p=128))
        maskb = None
        if flags["has_mask"]:
            maskb = const_p.tile([NTOK, 1], F32)
            nc.sync.dma_start(maskb[:], d["mask_bias"][:])

        # ===================================================================
        # Embedding (token-major) + LN, then transpose to feature-major
        # ===================================================================
        x = [act_p.tile([128, NTOK], F32, tag=f"x_{kh}", name=f"x_{kh}") for kh in range(KH)]
        with tc.tile_pool(name="emb", bufs=2) as emb_p:
            for tt_i in range(NTT):
                idx = misc_p.tile([128, 1], I32, tag="idx")
                nc.sync.dma_start(idx[:], d["ids"][tt_i * 128:(tt_i + 1) * 128, :])
                g = emb_p.tile([128, H], F32, tag="emb_g")
                nc.gpsimd.indirect_dma_start(
                    out=g[:], out_offset=None, in_=d["word_emb"][:],
                    in_offset=bass.IndirectOffsetOnAxis(ap=idx[:, :1], axis=0),
                )
                pt = emb_p.tile([128, H], F32, tag="pt")
                nc.sync.dma_start(pt[:], d["pt_emb"][tt_i * 128:(tt_i + 1) * 128, :])
                nc.vector.tensor_tensor(g[:], g[:], pt[:], op=OP.add)
                # token-major LN (per-partition stats)
                mu_t = stat_p.tile([128, 1], F32, tag="m1")
                nc.vector.tensor_reduce(mu_t[:], g[:], axis=AX.X, op=OP.add)
                scr = emb_p.tile([128, H], F32, tag="embsq")
                s2_t = stat_p.tile([128, 1], F32, tag="m2")
                nc.vector.scalar_tensor_tensor(
                    out=scr[:], in0=g[:], scalar=1.0, in1=g[:],
                    op0=OP.mult, op1=OP.mult, accum_out=s2_t[:])
                mu = stat_p.tile([128, 1], F32, tag="m3")
                nc.vector.tensor_scalar(out=mu[:], in0=mu_t[:], scalar1=1.0 / H,
                                        scalar2=None, op0=OP.mult)
                mu2 = stat_p.tile([128, 1], F32, tag="m4")
                nc.vector.tensor_tensor(mu2[:], mu[:], mu[:], op=OP.mult)
                var = stat_p.tile([128, 1], F32, tag="m5")
                nc.vector.scalar_tensor_tensor(
                    out=var[:], in0=s2_t[:], scalar=1.0 / H, in1=mu2[:],
                    op0=OP.mult, op1=OP.subtract)
                std = stat_p.tile([128, 1], F32, tag="m6")
                nc.scalar.activation(std[:], var[:], AF.Sqrt, bias=eps_col[:], scale=1.0)
                inv = stat_p.tile([128, 1], F32, tag="m7")
                nc.vector.reciprocal(inv[:], std[:])
                nmi = stat_p.tile([128, 1], F32, tag="m8")
                nc.vector.scalar_tensor_tensor(
                    out=nmi[:], in0=mu[:], scalar=-1.0, in1=inv[:],
                    op0=OP.mult, op1=OP.mult)
                nc.scalar.activation(g[:], g[:], AF.Identity, bias=nmi[:], scale=inv[:])
                if flags["has_emb_affine"]:
                    if tt_i == 0:
                        gt = const_p.tile([128, H], F32, tag="embg")
                        bt = const_p.tile([128, H], F32, tag="embb")
                        grow = misc_p.tile([1, H], F32, tag="grow")
                        brow = misc_p.tile([1, H], F32, tag="brow")
                        nc.sync.dma_start(grow[:], d["emb_ln"][0:1, :])
                        nc.sync.dma_start(brow[:], d["emb_ln"][1:2, :])
                        for half in range(2):
                            sl = slice(half * 384, (half + 1) * 384)
                            pg = ps_att.tile([128, 384], F32, space="PSUM", tag="att")
                            nc.tensor.matmul(pg[:], ones_row[0:1, :], grow[:, sl],
                                             start=True, stop=True)
                            nc.vector.tensor_copy(gt[:, sl], pg[:])
                            pb = ps_att.tile([128, 384], F32, space="PSUM", tag="att")
                            nc.tensor.matmul(pb[:], ones_row[0:1, :], brow[:, sl],
                                             start=True, stop=True)
                            nc.vector.tensor_copy(bt[:, sl], pb[:])
                    nc.vector.tensor_tensor(g[:], g[:], gt[:], op=OP.mult)
                    nc.vector.tensor_tensor(g[:], g[:], bt[:], op=OP.add)
                # transpose this token-tile into the feature-major x tiles
                for kh in range(KH):
                    pt_ps = ps_att.tile([128, 384], F32, space="PSUM", tag="att")
                    nc.tensor.transpose(pt_ps[:, :128],
                                        g[:, kh * 128:(kh + 1) * 128], ident[:])
                    nc.vector.tensor_copy(x[kh][:, tt_i * 128:(tt_i + 1) * 128],
                                          pt_ps[:, :128])

        # ===================================================================
        # Encoder layers
        # ===================================================================
        def layer_norm_fm(xin, lidx, which):
            """Feature-major LN over partition dim (H); in-place on xin."""
            ps_s1 = ps_sm.tile([1, NTOK], F32, space="PSUM", tag="sm1")
            ps_s2 = ps_sm.tile([11, NTOK], F32, space="PSUM", tag="sm2")
            for kh in range(KH):
                sq = sq_p.tile([128, NTOK], F32, tag="lnsq")
                nc.scalar.activation(sq[:], xin[kh][:], AF.Square)
                nc.tensor.matmul(ps_s1[:], ones_col[:], xin[kh][:],
                                 start=(kh == 0), stop=(kh == KH - 1))
                nc.tensor.matmul(ps_s2[:1], ones_col[:], sq[:],
                                 start=(kh == 0), stop=(kh == KH - 1))
            mu = stat_p.tile([1, NTOK], F32, tag="lnmu")
            nc.vector.tensor_scalar(out=mu[:], in0=ps_s1[:], scalar1=1.0 / H,
                                    scalar2=None, op0=OP.mult)
            mu2 = stat_p.tile([1, NTOK], F32, tag="lnmu2")
            nc.vector.tensor_tensor(mu2[:], mu[:], mu[:], op=OP.mult)
            var = stat_p.tile([1, NTOK], F32, tag="lnvar")
            nc.vector.scalar_tensor_tensor(
                out=var[:], in0=ps_s2[:1], scalar=1.0 / H, in1=mu2[:],
                op0=OP.mult, op1=OP.subtract)
            std = stat_p.tile([1, NTOK], F32, tag="lnstd")
            nc.scalar.activation(std[:], var[:], AF.Sqrt, bias=eps_col[:1, :], scale=1.0)
            inv = stat_p.tile([1, NTOK], F32, tag="lninv")
            nc.vector.reciprocal(inv[:], std[:])
            nmi = stat_p.tile([1, NTOK], F32, tag="lnnmi")
            nc.vector.scalar_tensor_tensor(
                out=nmi[:], in0=mu[:], scalar=-1.0, in1=inv[:],
                op0=OP.mult, op1=OP.mult)
            ps_a = ps_proj.tile([128, NTOK], F32, space="PSUM", tag="proj")
            ps_b = ps_proj.tile([128, NTOK], F32, space="PSUM", tag="proj")
            nc.tensor.matmul(ps_a[:], ones_row[0:1, :], inv[:], start=True, stop=True)
            nc.tensor.matmul(ps_b[:], ones_row[0:1, :], nmi[:], start=True, stop=True)
            for kh in range(KH):
                nc.vector.tensor_tensor(xin[kh][:], xin[kh][:], ps_a[:], op=OP.mult)
                nc.vector.tensor_tensor(xin[kh][:], xin[kh][:], ps_b[:], op=OP.add)
                if flags["has_ln_affine"]:
                    gcol = misc_p.tile([128, 1], F32, tag="lngc")
                    bcol = misc_p.tile([128, 1], F32, tag="lnbc")
                    nc.sync.dma_start(
                        gcol[:], d["ln_gb"][lidx, which, 0, kh * 128:(kh + 1) * 128, None])
                    nc.sync.dma_start(
                        bcol[:], d["ln_gb"][lidx, which, 1, kh * 128:(kh + 1) * 128, None])
                    nc.vector.tensor_scalar(out=xin[kh][:], in0=xin[kh][:],
                                            scalar1=gcol[:], scalar2=bcol[:],
                                            op0=OP.mult, op1=OP.add)
            return xin

        with ExitStack() as wstack:
            w_p = wstack.enter_context(tc.tile_pool(name="wslab", bufs=2))
            w2_p = wstack.enter_context(tc.tile_pool(name="w2slab", bufs=2))
            w1_p = wstack.enter_context(tc.tile_pool(name="w1slab", bufs=3))
            g_p = wstack.enter_context(tc.tile_pool(name="gelu", bufs=1))

            for lidx in range(N_LAYERS):
                bcols = None
                if flags["has_bias"]:
                    bcols = misc_p.tile([128, 6, KF], F32, tag="bcols")
                    nc.sync.dma_start(
                        bcols[:], d["biases"][lidx].rearrange("r (ko p) -> p r ko", p=128))

                # ---- Q, K projections (feature-major) ----
                qT, kT = [], []
                for wi, which, outl in ((0, "q", qT), (1, "k", kT)):
                    ws = w_p.tile([128, KH, H], F32, tag="wqkvo")
                    nc.sync.dma_start(
                        ws[:], d["w_qkvo"][lidx, wi].rearrange("(ko p) m -> p ko m", p=128))
                    for mh in range(KH):
                        ps = ps_proj.tile([128, NTOK], F32, space="PSUM", tag="proj")
                        for kh in range(KH):
                            nc.tensor.matmul(ps[:], ws[:, kh, mh * 128:(mh + 1) * 128],
                                             x[kh][:],
                                             start=(kh == 0), stop=(kh == KH - 1))
                        o = qkv_p.tile([128, NTOK], F32, tag=f"{which}T_{mh}")
                        if flags["has_bias"]:
                            nc.vector.tensor_scalar(
                                out=o[:], in0=ps[:],
                                scalar1=bcols[:, wi, mh:mh + 1],
                                scalar2=None, op0=OP.add)
                        else:
                            nc.vector.tensor_copy(o[:], ps[:])
                        outl.append(o)

                # ---- V projection (token-major: stationary x, moving Wv) ----
                wv = w_p.tile([128, KH, H], F32, tag="wqkvo")
                nc.sync.dma_start(
                    wv[:], d["w_qkvo"][lidx, 2].rearrange("(ko p) m -> p ko m", p=128))
                bv_bc = None
                if flags["has_bias"]:
                    bv_bc = misc_p.tile([128, H], F32, tag="bvbc")
                    bvrow = misc_p.tile([1, H], F32, tag="bvrow")
                    nc.sync.dma_start(bvrow[:], d["biases"][lidx, 5:6, :H])
                    for half in range(2):
                        sl = slice(half * 384, (half + 1) * 384)
                        psb = ps_att.tile([128, 384], F32, space="PSUM", tag="att")
                        nc.tensor.matmul(psb[:], ones_row[0:1, :], bvrow[:, sl],
                                         start=True, stop=True)
                        nc.vector.tensor_copy(bv_bc[:, sl], psb[:])
                v_tm = []
                for mt in range(NTT):
                    o = qkv_p.tile([128, H], F32, tag=f"v_{mt}")
                    for half in range(2):
                        sl = slice(half * 384, (half + 1) * 384)
                        ps = ps_att.tile([128, 384], F32, space="PSUM", tag="att")
                        for kh in range(KH):
                            nc.tensor.matmul(ps[:], x[kh][:, mt * 128:(mt + 1) * 128],
                                             wv[:, kh, sl],
                                             start=(kh == 0), stop=(kh == KH - 1))
                        if flags["has_bias"]:
                            nc.vector.tensor_tensor(o[:, sl], ps[:], bv_bc[:, sl], op=OP.add)
                        else:
                            nc.vector.tensor_copy(o[:, sl], ps[:])
                    v_tm.append(o)

                # ---- attention per (seq, head) ----
                ctxT = [qkv_p.tile([128, NTOK], F32, tag=f"ctxT_{mh}", name=f"ctxT_{mh}")
                        for mh in range(KH)]
                for bseq in range(SEQ_PER_CORE):
                    for hh in range(NHEAD):
                        tile_i, row0 = hh // 2, (hh % 2) * 64
                        qs = qT[tile_i][row0:row0 + 64, bseq * T:(bseq + 1) * T]
                        exps = []
                        for kc in range(2):
                            ks = kT[tile_i][row0:row0 + 64,
                                            bseq * T + kc * 128: bseq * T + (kc + 1) * 128]
                            ps_s = ps_att.tile([128, 384], F32, space="PSUM", tag="att")
                            nc.tensor.matmul(ps_s[:, :T], ks, qs, start=True, stop=True)
                            e = exp_p.tile([128, T], F32, tag="expT")
                            if flags["has_mask"]:
                                nc.scalar.activation(
                                    e[:], ps_s[:, :T], AF.Exp,
                                    bias=maskb[bseq * T + kc * 128:
                                               bseq * T + (kc + 1) * 128, :],
                                    scale=0.125)
                            else:
                                nc.scalar.activation(e[:], ps_s[:, :T], AF.Exp,
                                                     bias=0.0, scale=0.125)
                            exps.append(e)
                        ps_den = ps_sm.tile([1, T], F32, space="PSUM", tag="sm1")
                        for kc in range(2):
                            nc.tensor.matmul(ps_den[:], ones_col[:], exps[kc][:],
                                             start=(kc == 0), stop=(kc == 1))
                        ps_ctx = ps_att.tile([128, 384], F32, space="PSUM", tag="att")
                        for kc in range(2):
                            vs = v_tm[bseq * 2 + kc][:, hh * 64:(hh + 1) * 64]
                            nc.tensor.matmul(ps_ctx[:64, :T], vs, exps[kc][:],
                                             start=(kc == 0), stop=(kc == 1))
                        recip = misc_p.tile([1, T], F32, tag="recip")
                        nc.vector.reciprocal(recip[:], ps_den[:])
                        ps_rb = ps_att.tile([128, 384], F32, space="PSUM", tag="att")
                        nc.tensor.matmul(ps_rb[:64, :T], ones_row[0:1, :64], recip[:],
                                         start=True, stop=True)
                        rb_sb = misc_p.tile([64, T], F32, tag="rb_sb")
                        nc.vector.tensor_copy(rb_sb[:], ps_rb[:64, :T])
                        nc.vector.tensor_tensor(
                            ctxT[tile_i][row0:row0 + 64, bseq * T:(bseq + 1) * T],
                            ps_ctx[:64, :T], rb_sb[:], op=OP.mult)

                # ---- O-projection + residual (in-place into x) ----
                wo = w_p.tile([128, KH, H], F32, tag="wqkvo")
                nc.sync.dma_start(
                    wo[:], d["w_qkvo"][lidx, 3].rearrange("(ko p) m -> p ko m", p=128))
                for mh in range(KH):
                    ps = ps_proj.tile([128, NTOK], F32, space="PSUM", tag="proj")
                    for kh in range(KH):
                        nc.tensor.matmul(ps[:], wo[:, kh, mh * 128:(mh + 1) * 128],
                                         ctxT[kh][:],
                                         start=(kh == 0), stop=(kh == KH - 1))
                    if flags["has_bias"]:
                        nc.vector.scalar_tensor_tensor(
                            out=x[mh][:], in0=ps[:], scalar=bcols[:, 2, mh:mh + 1],
                            in1=x[mh][:], op0=OP.add, op1=OP.add)
                    else:
                        nc.vector.tensor_tensor(x[mh][:], ps[:], x[mh][:], op=OP.add)
                x = layer_norm_fm(x, lidx, 0)

                # ---- FFN (x2 := x after LN1; gelu -> gT; FFN2 + residual) ----
                gT = []
                for mf in range(KF):
                    w1s = w1_p.tile([128, KH, 128], F32, tag="w1s")
                    nc.sync.dma_start(
                        w1s[:], d["w_ffn1"][lidx, mf].rearrange("(ko p) m -> p ko m", p=128))
                    ps = ps_proj.tile([128, NTOK], F32, space="PSUM", tag="proj")
                    for kh in range(KH):
                        nc.tensor.matmul(ps[:], w1s[:, kh, :], x[kh][:],
                                         start=(kh == 0), stop=(kh == KH - 1))
                    g = g_p.tile([128, NTOK], F32, tag=f"gT_{mf}")
                    if flags["has_bias"]:
                        nc.scalar.activation(g[:], ps[:], AF.Gelu,
                                             bias=bcols[:, 3, mf:mf + 1], scale=1.0)
                    else:
                        nc.scalar.activation(g[:], ps[:], AF.Gelu, bias=0.0, scale=1.0)
                    gT.append(g)
                for mh in range(KH):
                    ps = ps_proj.tile([128, NTOK], F32, space="PSUM", tag="proj")
                    for half in range(2):
                        w2s = w2_p.tile([128, KF // 2, 128], F32, tag="w2s")
                        nc.sync.dma_start(
                            w2s[:], d["w_ffn2"][lidx, mh,
                                                half * 1536:(half + 1) * 1536]
                            .rearrange("(ko p) m -> p ko m", p=128))
                        for kf2 in range(KF // 2):
                            kf = half * (KF // 2) + kf2
                            nc.tensor.matmul(ps[:], w2s[:, kf2, :], gT[kf][:],
                                             start=(kf == 0), stop=(kf == KF - 1))
                    if flags["has_bias"]:
                        nc.vector.scalar_tensor_tensor(
                            out=x[mh][:], in0=ps[:], scalar=bcols[:, 4, mh:mh + 1],
                            in1=x[mh][:], op0=OP.add, op1=OP.add)
                    else:
                        nc.vector.tensor_tensor(x[mh][:], ps[:], x[mh][:], op=OP.add)
                x = layer_norm_fm(x, lidx, 1)

        # ===================================================================
        # feats projection + featsV rearrange + Viterbi
        # ===================================================================
        with tc.tile_pool(name="vit", bufs=1) as vit_p:
            ps_f = ps_sm.tile([11, NTOK], F32, space="PSUM", tag="sm2")
            for kh in range(KH):
                nc.tensor.matmul(ps_f[:], wout[:, kh, :], x[kh][:],
                                 start=(kh == 0), stop=(kh == KH - 1))
            featsT = vit_p.tile([NL, NTOK], F32, tag="featsT")
            if flags["has_bout"]:
                bout = misc_p.tile([NL, 1], F32, tag="bout")
                nc.sync.dma_start(bout[:], d["b_out"][:])
                nc.vector.tensor_scalar(out=featsT[:], in0=ps_f[:], scalar1=bout[:],
                                        scalar2=None, op0=OP.add)
            else:
                nc.vector.tensor_copy(featsT[:], ps_f[:])

            featsV = vit_p.tile([2, T * NL], F32, tag="featsV")
            for n in range(NL):
                src = featsT[n:n + 1, :].rearrange("p (b t) -> p b t", b=2)
                dst = featsV[:].rearrange("p (t n) -> p t n", n=NL)[:, :, n]
                nc.sync.dma_start(dst, src)

            # ---- forward DP ----
            delta = vit_p.tile([2, NL], F32, tag="delta")
            nc.sync.dma_start(delta[:], d["vit_init"][:])
            psiR = vit_p.tile([2, (T - 1) * NL], F32, tag="psiR")
            s_t = vit_p.tile([2, NL * NL], F32, tag="s_t")
            eq_t = vit_p.tile([2, NL * NL], F32, tag="eq_t")
            pr_t = vit_p.tile([2, NL * NL], F32, tag="pr_t")
            m_t = vit_p.tile([2, NL], F32, tag="m_t")

            t3 = lambda ap: ap.rearrange("b (n p) -> b n p", n=NL)
            for t in range(1, T):
                nc.vector.tensor_tensor(
                    t3(s_t[:]), t3(trans2[:]),
                    delta[:][:, None, :].broadcast_to((2, NL, NL)), op=OP.add)
                nc.vector.tensor_reduce(m_t[:], t3(s_t[:]), axis=AX.X, op=OP.max)
                nc.vector.tensor_tensor(
                    t3(eq_t[:]), t3(s_t[:]), m_t[:].to_broadcast([2, NL, NL]),
                    op=OP.is_equal)
                nc.vector.tensor_tensor(pr_t[:], eq_t[:], rev121[:], op=OP.mult)
                nc.vector.tensor_reduce(
                    psiR[:, (t - 1) * NL: t * NL], t3(pr_t[:]), axis=AX.X, op=OP.max)
                nc.vector.tensor_tensor(
                    delta[:], m_t[:], featsV[:, t * NL:(t + 1) * NL], op=OP.add)

            # ---- final argmax + score ----
            score_f = vit_p.tile([2, 1], F32, tag="score")
            nc.vector.tensor_reduce(score_f[:], delta[:], axis=AX.X, op=OP.max)
            nc.sync.dma_start(d["score_out"][:], score_f[:])
            eqf = vit_p.tile([2, NL], F32, tag="eqf")
            nc.vector.tensor_scalar(out=eqf[:], in0=delta[:], scalar1=score_f[:],
                                    scalar2=None, op0=OP.is_equal)
            prf = vit_p.tile([2, NL], F32, tag="prf")
            nc.vector.tensor_tensor(prf[:], eqf[:], rev11[:], op=OP.mult)
            selR = vit_p.tile([2, 1], F32, tag="selR")
            nc.vector.tensor_reduce(selR[:], prf[:], axis=AX.X, op=OP.max)

            path_f = vit_p.tile([2, T], F32, tag="path_f")
            onehot = vit_p.tile([2, NL], F32, tag="onehot")
            scrv = vit_p.tile([2, NL], F32, tag="scrv")
            nc.vector.tensor_scalar(out=path_f[:, T - 1:T], in0=selR[:], scalar1=-1.0,
                                    scalar2=10.0, op0=OP.mult, op1=OP.add)
            nc.vector.tensor_scalar(out=onehot[:], in0=rev11[:], scalar1=selR[:],
                                    scalar2=None, op0=OP.is_equal)
            for t in range(T - 1, 0, -1):
                nc.vector.scalar_tensor_tensor(
                    out=scrv[:], in0=psiR[:, (t - 1) * NL: t * NL], scalar=1.0,
                    in1=onehot[:], op0=OP.mult, op1=OP.mult, accum_out=selR[:])
                nc.vector.tensor_scalar(out=path_f[:, t - 1:t], in0=selR[:],
                                        scalar1=-1.0, scalar2=10.0,
                                        op0=OP.mult, op1=OP.add)
                if t > 1:
                    nc.vector.tensor_scalar(out=onehot[:], in0=rev11[:], scalar1=selR[:],
                                            scalar2=None, op0=OP.is_equal)
            path_i = vit_p.tile([2, T], I32, tag="path_i")
            nc.vector.tensor_copy(path_i[:], path_f[:])
            nc.sync.dma_start(d["path_out"][:], path_i[:])


# ---------------------------------------------------------------------------
# host side
# ---------------------------------------------------------------------------

_CACHE = {}


def _prepare(params):
    p = {k: _np(v) for k, v in params.items() if k != "layers"}
    lay = {k: _np(v) for k, v in params["layers"].items()}
    L = N_LAYERS

    flags = {
        "has_bias": any(np.any(lay[b]) for b in ("bq", "bk", "bv", "bo", "b1", "b2")),
        "has_ln_affine": bool(not np.all(lay["ln1_g"] == 1) or np.any(lay["ln1_b"])
                              or not np.all(lay["ln2_g"] == 1) or np.any(lay["ln2_b"])),
        "has_emb_affine": bool(not np.all(p["emb_ln_g"] == 1) or np.any(p["emb_ln_b"])),
        "has_bout": bool(np.any(p["out_b"])),
    }

    w_qkvo = np.stack([lay["Wq"][:L], lay["Wk"][:L], lay["Wv"][:L], lay["Wo"][:L]],
                      axis=1)
    w_ffn1 = np.ascontiguousarray(
        lay["W1"][:L].reshape(L, H, KF, 128).transpose(0, 2, 1, 3))
    w_ffn2 = np.ascontiguousarray(
        lay["W2"][:L].reshape(L, FF, KH, 128).transpose(0, 2, 1, 3))
    biases = np.zeros((L, 6, FF), np.float32)
    biases[:, 0, :H] = lay["bq"][:L]
    biases[:, 1, :H] = lay["bk"][:L]
    biases[:, 2, :H] = lay["bo"][:L]
    biases[:, 3, :] = lay["b1"][:L]
    biases[:, 4, :H] = lay["b2"][:L]
    biases[:, 5, :H] = lay["bv"][:L]
    ln_gb = np.stack([
        np.stack([lay["ln1_g"][:L], lay["ln1_b"][:L]], axis=1),
        np.stack([lay["ln2_g"][:L], lay["ln2_b"][:L]], axis=1)], axis=1)

    trans = p["transitions"].astype(np.float32)
    trans2 = np.broadcast_to(trans.reshape(1, NL * NL), (2, NL * NL))
    rev = (10.0 - np.arange(NL)).astype(np.float32)
    rev121 = np.broadcast_to(np.tile(rev, NL).reshape(1, NL * NL), (2, NL * NL))
    rev11 = np.broadcast_to(rev.reshape(1, NL), (2, NL))
    vinit = np.full((2, NL), NEG, np.float32)
    vinit[:, START_ID] = 0.0

    shared = {
        "word_emb": p["word_emb"],
        "w_qkvo": w_qkvo, "w_ffn1": w_ffn1, "w_ffn2": w_ffn2,
        "biases": biases, "ln_gb": ln_gb,
        "w_out": p["out_W"],
        "b_out": p["out_b"].reshape(NL, 1),
        "emb_ln": np.stack([p["emb_ln_g"], p["emb_ln_b"]]),
        "trans2": trans2, "rev121": rev121, "rev11": rev11,
        "vit_init": vinit,
        "ident": np.eye(128, dtype=np.float32),
        "ones_col": np.ones((128, 1), np.float32),
        "ones_row": np.ones((1, 128), np.float32),
    }
    shared = {k: np.ascontiguousarray(v.astype(np.float32, copy=False))
              for k, v in shared.items()}
    return shared, flags, p


def kernel(input_ids, segment_ids, input_mask, params):
    input_ids = _np(input_ids)
    segment_ids = _np(segment_ids)
    input_mask = _np(input_mask)
    shared, flags, p = _prepare(params)
    flags["has_mask"] = bool(np.any(input_mask != 1))

    key = (N_LAYERS, tuple(sorted(flags.items())))
    if key not in _CACHE:
        _CACHE[key] = build_module(flags)
    nc = _CACHE[key]

    pos_type = p["pos_emb"][:T][None, :, :] + p["type_emb"][segment_ids]  # [B,T,H]
    mask_bias = ((1.0 - input_mask.astype(np.float32)) * NEG)             # [B,T]

    in_maps = []
    for c in range(NCORES):
        sl = slice(c * SEQ_PER_CORE, (c + 1) * SEQ_PER_CORE)
        m = dict(shared)
        m["ids"] = np.ascontiguousarray(input_ids[sl].reshape(NTOK, 1).astype(np.int32))
        m["pt_emb"] = np.ascontiguousarray(
            pos_type[sl].reshape(NTOK, H).astype(np.float32))
        m["mask_bias"] = np.ascontiguousarray(
            mask_bias[sl].reshape(NTOK, 1).astype(np.float32))
        in_maps.append(m)

    res = run_bass_kernel_spmd(
        nc, in_maps, core_ids=list(range(NCORES)),
        trace=bool(int(os.environ.get("KERNEL_TRACE", "0"))))
    score = np.concatenate([r["score_out"].reshape(SEQ_PER_CORE) for r in res.results])
    path = np.concatenate([r["path_out"] for r in res.results], axis=0)
    kernel.last_result = res
    return score.astype(np.float32), path.astype(np.int32)


# revision 7
# speedup vs baseline: 1.1587x; 1.1587x over previous
"""BERT-base + CRF Viterbi decode on 8 Trainium2 NeuronCores.

Sharding: data-parallel over batch (16 seqs -> 2 per core). Each core runs
the full 12-layer encoder on its 512 tokens (feature-major activations,
fp32 matmuls for Viterbi argmax exactness) plus the CRF Viterbi DP on DVE.

Self-contained: hardcodes all shapes from the problem spec.
"""
import os
from contextlib import ExitStack

import numpy as np

import concourse.bass as bass
import concourse.mybir as mybir
import concourse.tile as tile
from concourse import bacc
from concourse.bass_utils import run_bass_kernel_spmd

F32 = mybir.dt.float32
I32 = mybir.dt.int32
AF = mybir.ActivationFunctionType
OP = mybir.AluOpType
AX = mybir.AxisListType

V, TMAX, H, L_FULL, FF, NHEAD, DH = 30522, 512, 768, 12, 3072, 12, 64
NL, START_ID, STOP_ID = 11, 9, 10
B, T = 16, 256
NEG = -10000.0
NCORES = 8
SEQ_PER_CORE = B // NCORES          # 2
NTOK = SEQ_PER_CORE * T             # 512
KH = H // 128                       # 6 h-tiles
KF = FF // 128                      # 24 f-tiles
NTT = NTOK // 128                   # 4 token-tiles
EPS = 1e-12

N_LAYERS = int(os.environ.get("KERNEL_N_LAYERS", str(L_FULL)))
N_REPEAT = int(os.environ.get("KERNEL_REPEAT", "1"))


def _np(x):
    return np.ascontiguousarray(np.asarray(x))


# ---------------------------------------------------------------------------
# device program
# ---------------------------------------------------------------------------

def build_module(flags):
    nc = bacc.Bacc(None, target_bir_lowering=False, debug=False)

    d = {}
    def inp(name, shape, dt=F32):
        d[name] = nc.dram_tensor(name, shape, dt, kind="ExternalInput")
        return d[name]

    inp("ids", [NTOK, 1], I32)
    inp("word_emb", [V, H])
    inp("pt_emb", [NTOK, H])
    inp("w_qkvo", [N_LAYERS, 4, H, H])
    inp("w_ffn1", [N_LAYERS, KF, H, 128])
    inp("w_ffn2", [N_LAYERS, KH, FF, 128])
    inp("biases", [N_LAYERS, 6, FF])   # rows: bq bk bo b1 b2 bv
    inp("ln_gb", [N_LAYERS, 2, 2, H])
    inp("w_out", [H, NL])
    inp("b_out", [NL, 1])
    inp("emb_ln", [2, H])
    inp("mask_bias", [NTOK, 1])
    inp("trans2", [2, NL * NL])
    inp("rev121", [2, NL * NL])
    inp("rev11", [2, NL])
    inp("vit_init", [2, NL])
    inp("ident", [128, 128])
    inp("ones_col", [128, 1])
    inp("ones_row", [1, 128])

    d["score_out"] = nc.dram_tensor("score_out", [2, 1], F32, kind="ExternalOutput")
    d["path_out"] = nc.dram_tensor("path_out", [2, T], I32, kind="ExternalOutput")

    with tile.TileContext(nc) as tc:
        for _rep in range(N_REPEAT):
            _emit(nc, tc, flags, d)
    nc.finalize()
    return nc


def _emit(nc, tc, flags, d):
    with ExitStack() as top:
        # long-lived pools
        const_p = top.enter_context(tc.tile_pool(name="const", bufs=1))
        act_p = top.enter_context(tc.tile_pool(name="act", bufs=1))
        qkv_p = top.enter_context(tc.tile_pool(name="qkv", bufs=1))
        exp_p = top.enter_context(tc.tile_pool(name="exp", bufs=4))
        misc_p = top.enter_context(tc.tile_pool(name="misc", bufs=2))
        stat_p = top.enter_context(tc.tile_pool(name="stat", bufs=2))
        sq_p = top.enter_context(tc.tile_pool(name="sq", bufs=2))
        ps_proj = top.enter_context(tc.tile_pool(name="ps_proj", bufs=3, space="PSUM"))
        ps_att = top.enter_context(tc.tile_pool(name="ps_att", bufs=3, space="PSUM"))
        ps_sm = top.enter_context(tc.tile_pool(name="ps_sm", bufs=1, space="PSUM"))

        # ---- constants ----
        ident = const_p.tile([128, 128], F32)
        nc.sync.dma_start(ident[:], d["ident"][:])
        ones_col = const_p.tile([128, 1], F32)
        nc.sync.dma_start(ones_col[:], d["ones_col"][:])
        ones_row = const_p.tile([1, 128], F32)
        nc.sync.dma_start(ones_row[:], d["ones_row"][:])
        trans2 = const_p.tile([2, NL * NL], F32)
        nc.sync.dma_start(trans2[:], d["trans2"][:])
        rev121 = const_p.tile([2, NL * NL], F32)
        nc.sync.dma_start(rev121[:], d["rev121"][:])
        rev11 = const_p.tile([2, NL], F32)
        nc.sync.dma_start(rev11[:], d["rev11"][:])
        eps_col = const_p.tile([128, 1], F32)
        nc.vector.memset(eps_col[:], EPS)
        wout = const_p.tile([128, KH, NL], F32)
        nc.sync.dma_start(wout[:], d["w_out"].rearrange("(ko p) m -> p ko m", p=128))
        maskb = None
        if flags["has_mask"]:
            maskb = const_p.tile([NTOK, 1], F32)
            nc.sync.dma_start(maskb[:], d["mask_bias"][:])

        # ===================================================================
        # Embedding (token-major) + LN, then transpose to feature-major
        # ===================================================================
        x = [act_p.tile([128, NTOK], F32, tag=f"x_{kh}", name=f"x_{kh}") for kh in range(KH)]
        with tc.tile_pool(name="emb", bufs=2) as emb_p:
            for tt_i in range(NTT):
                idx = misc_p.tile([128, 1], I32, tag="idx")
                nc.sync.dma_start(idx[:], d["ids"][tt_i * 128:(tt_i + 1) * 128, :])
                g = emb_p.tile([128, H], F32, tag="emb_g")
                nc.gpsimd.indirect_dma_start(
                    out=g[:], out_offset=None, in_=d["word_emb"][:],
                    in_offset=bass.IndirectOffsetOnAxis(ap=idx[:, :1], axis=0),
                )
                pt = emb_p.tile([128, H], F32, tag="pt")
                nc.sync.dma_start(pt[:], d["pt_emb"][tt_i * 128:(tt_i + 1) * 128, :])
                nc.vector.tensor_tensor(g[:], g[:], pt[:], op=OP.add)
                # token-major LN (per-partition stats)
                mu_t = stat_p.tile([128, 1], F32, tag="m1")
                nc.vector.tensor_reduce(mu_t[:], g[:], axis=AX.X, op=OP.add)
                scr = emb_p.tile([128, H], F32, tag="embsq")
                s2_t = stat_p.tile([128, 1], F32, tag="m2")
                nc.vector.scalar_tensor_tensor(
                    out=scr[:], in0=g[:], scalar=1.0, in1=g[:],
                    op0=OP.mult, op1=OP.mult, accum_out=s2_t[:])
                mu = stat_p.tile([128, 1], F32, tag="m3")
                nc.vector.tensor_scalar(out=mu[:], in0=mu_t[:], scalar1=1.0 / H,
                                        scalar2=None, op0=OP.mult)
                mu2 = stat_p.tile([128, 1], F32, tag="m4")
                nc.vector.tensor_tensor(mu2[:], mu[:], mu[:], op=OP.mult)
                var = stat_p.tile([128, 1], F32, tag="m5")
                nc.vector.scalar_tensor_tensor(
                    out=var[:], in0=s2_t[:], scalar=1.0 / H, in1=mu2[:],
                    op0=OP.mult, op1=OP.subtract)
                std = stat_p.tile([128, 1], F32, tag="m6")
                nc.scalar.activation(std[:], var[:], AF.Sqrt, bias=eps_col[:], scale=1.0)
                inv = stat_p.tile([128, 1], F32, tag="m7")
                nc.vector.reciprocal(inv[:], std[:])
                nmi = stat_p.tile([128, 1], F32, tag="m8")
                nc.vector.scalar_tensor_tensor(
                    out=nmi[:], in0=mu[:], scalar=-1.0, in1=inv[:],
                    op0=OP.mult, op1=OP.mult)
                nc.scalar.activation(g[:], g[:], AF.Identity, bias=nmi[:], scale=inv[:])
                if flags["has_emb_affine"]:
                    if tt_i == 0:
                        gt = const_p.tile([128, H], F32, tag="embg")
                        bt = const_p.tile([128, H], F32, tag="embb")
                        grow = misc_p.tile([1, H], F32, tag="grow")
                        brow = misc_p.tile([1, H], F32, tag="brow")
                        nc.sync.dma_start(grow[:], d["emb_ln"][0:1, :])
                        nc.sync.dma_start(brow[:], d["emb_ln"][1:2, :])
                        for half in range(2):
                            sl = slice(half * 384, (half + 1) * 384)
                            pg = ps_att.tile([128, 384], F32, space="PSUM", tag="att")
                            nc.tensor.matmul(pg[:], ones_row[0:1, :], grow[:, sl],
                                             start=True, stop=True)
                            nc.vector.tensor_copy(gt[:, sl], pg[:])
                            pb = ps_att.tile([128, 384], F32, space="PSUM", tag="att")
                            nc.tensor.matmul(pb[:], ones_row[0:1, :], brow[:, sl],
                                             start=True, stop=True)
                            nc.vector.tensor_copy(bt[:, sl], pb[:])
                    nc.vector.tensor_tensor(g[:], g[:], gt[:], op=OP.mult)
                    nc.vector.tensor_tensor(g[:], g[:], bt[:], op=OP.add)
                # transpose this token-tile into the feature-major x tiles
                for kh in range(KH):
                    pt_ps = ps_att.tile([128, 384], F32, space="PSUM", tag="att")
                    nc.tensor.transpose(pt_ps[:, :128],
                                        g[:, kh * 128:(kh + 1) * 128], ident[:])
                    nc.vector.tensor_copy(x[kh][:, tt_i * 128:(tt_i + 1) * 128],
                                          pt_ps[:, :128])

        # ===================================================================
        # Encoder layers
        # ===================================================================
        def layer_norm_fm(xin, lidx, which):
            """Feature-major LN over partition dim (H); in-place on xin."""
            ps_s1 = ps_sm.tile([1, NTOK], F32, space="PSUM", tag="sm1")
            ps_s2 = ps_sm.tile([11, NTOK], F32, space="PSUM", tag="sm2")
            for kh in range(KH):
                sq = sq_p.tile([128, NTOK], F32, tag="lnsq")
                nc.scalar.activation(sq[:], xin[kh][:], AF.Square)
                nc.tensor.matmul(ps_s1[:], ones_col[:], xin[kh][:],
                                 start=(kh == 0), stop=(kh == KH - 1))
                nc.tensor.matmul(ps_s2[:1], ones_col[:], sq[:],
                                 start=(kh == 0), stop=(kh == KH - 1))
            mu = stat_p.tile([1, NTOK], F32, tag="lnmu")
            nc.vector.tensor_scalar(out=mu[:], in0=ps_s1[:], scalar1=1.0 / H,
                                    scalar2=None, op0=OP.mult)
            mu2 = stat_p.tile([1, NTOK], F32, tag="lnmu2")
            nc.vector.tensor_tensor(mu2[:], mu[:], mu[:], op=OP.mult)
            var = stat_p.tile([1, NTOK], F32, tag="lnvar")
            nc.vector.scalar_tensor_tensor(
                out=var[:], in0=ps_s2[:1], scalar=1.0 / H, in1=mu2[:],
                op0=OP.mult, op1=OP.subtract)
            std = stat_p.tile([1, NTOK], F32, tag="lnstd")
            nc.scalar.activation(std[:], var[:], AF.Sqrt, bias=eps_col[:1, :], scale=1.0)
            inv = stat_p.tile([1, NTOK], F32, tag="lninv")
            nc.vector.reciprocal(inv[:], std[:])
            nmi = stat_p.tile([1, NTOK], F32, tag="lnnmi")
            nc.vector.scalar_tensor_tensor(
                out=nmi[:], in0=mu[:], scalar=-1.0, in1=inv[:],
                op0=OP.mult, op1=OP.mult)
            ps_a = ps_proj.tile([128, NTOK], F32, space="PSUM", tag="proj")
            ps_b = ps_proj.tile([128, NTOK], F32, space="PSUM", tag="proj")
            nc.tensor.matmul(ps_a[:], ones_row[0:1, :], inv[:], start=True, stop=True)
            nc.tensor.matmul(ps_b[:], ones_row[0:1, :], nmi[:], start=True, stop=True)
            for kh in range(KH):
                nc.vector.tensor_tensor(xin[kh][:], xin[kh][:], ps_a[:], op=OP.mult)
                nc.vector.tensor_tensor(xin[kh][:], xin[kh][:], ps_b[:], op=OP.add)
                if flags["has_ln_affine"]:
                    gcol = misc_p.tile([128, 1], F32, tag="lngc")
                    bcol = misc_p.tile([128, 1], F32, tag="lnbc")
                    nc.sync.dma_start(
                        gcol[:], d["ln_gb"][lidx, which, 0, kh * 128:(kh + 1) * 128, None])
                    nc.sync.dma_start(
                        bcol[:], d["ln_gb"][lidx, which, 1, kh * 128:(kh + 1) * 128, None])
                    nc.vector.tensor_scalar(out=xin[kh][:], in0=xin[kh][:],
                                            scalar1=gcol[:], scalar2=bcol[:],
                                            op0=OP.mult, op1=OP.add)
            return xin

        with ExitStack() as wstack:
            w_p = wstack.enter_context(tc.tile_pool(name="wslab", bufs=2))
            w2_p = wstack.enter_context(tc.tile_pool(name="w2slab", bufs=2))
            w1_p = wstack.enter_context(tc.tile_pool(name="w1slab", bufs=3))
            g_p = wstack.enter_context(tc.tile_pool(name="gelu", bufs=1))

            for lidx in range(N_LAYERS):
                bcols = None
                if flags["has_bias"]:
                    bcols = misc_p.tile([128, 6, KF], F32, tag="bcols")
                    nc.sync.dma_start(
                        bcols[:], d["biases"][lidx].rearrange("r (ko p) -> p r ko", p=128))

                # ---- Q, K projections (feature-major) ----
                qT, kT = [], []
                for wi, which, outl in ((0, "q", qT), (1, "k", kT)):
                    ws = w_p.tile([128, KH, H], F32, tag="wqkvo")
                    nc.sync.dma_start(
                        ws[:], d["w_qkvo"][lidx, wi].rearrange("(ko p) m -> p ko m", p=128))
                    for mh in range(KH):
                        ps = ps_proj.tile([128, NTOK], F32, space="PSUM", tag="proj")
                        for kh in range(KH):
                            nc.tensor.matmul(ps[:], ws[:, kh, mh * 128:(mh + 1) * 128],
                                             x[kh][:],
                                             start=(kh == 0), stop=(kh == KH - 1))
                        o = qkv_p.tile([128, NTOK], F32, tag=f"{which}T_{mh}")
                        if flags["has_bias"]:
                            nc.vector.tensor_scalar(
                                out=o[:], in0=ps[:],
                                scalar1=bcols[:, wi, mh:mh + 1],
                                scalar2=None, op0=OP.add)
                        else:
                            nc.vector.tensor_copy(o[:], ps[:])
                        outl.append(o)

                # ---- V projection (token-major: stationary x, moving Wv) ----
                wv = w_p.tile([128, KH, H], F32, tag="wqkvo")
                nc.sync.dma_start(
                    wv[:], d["w_qkvo"][lidx, 2].rearrange("(ko p) m -> p ko m", p=128))
                bv_bc = None
                if flags["has_bias"]:
                    bv_bc = misc_p.tile([128, H], F32, tag="bvbc")
                    bvrow = misc_p.tile([1, H], F32, tag="bvrow")
                    nc.sync.dma_start(bvrow[:], d["biases"][lidx, 5:6, :H])
                    for half in range(2):
                        sl = slice(half * 384, (half + 1) * 384)
                        psb = ps_att.tile([128, 384], F32, space="PSUM", tag="att")
                        nc.tensor.matmul(psb[:], ones_row[0:1, :], bvrow[:, sl],
                                         start=True, stop=True)
                        nc.vector.tensor_copy(bv_bc[:, sl], psb[:])
                v_tm = []
                for mt in range(NTT):
                    o = qkv_p.tile([128, H], F32, tag=f"v_{mt}")
                    for half in range(2):
                        sl = slice(half * 384, (half + 1) * 384)
                        ps = ps_att.tile([128, 384], F32, space="PSUM", tag="att")
                        for kh in range(KH):
                            nc.tensor.matmul(ps[:], x[kh][:, mt * 128:(mt + 1) * 128],
                                             wv[:, kh, sl],
                                             start=(kh == 0), stop=(kh == KH - 1))
                        if flags["has_bias"]:
                            nc.vector.tensor_tensor(o[:, sl], ps[:], bv_bc[:, sl], op=OP.add)
                        else:
                            nc.vector.tensor_copy(o[:, sl], ps[:])
                    v_tm.append(o)

                # ---- attention per (seq, head) ----
                ctxT = [qkv_p.tile([128, NTOK], F32, tag=f"ctxT_{mh}", name=f"ctxT_{mh}")
                        for mh in range(KH)]
                for bseq in range(SEQ_PER_CORE):
                    for hh in range(NHEAD):
                        tile_i, row0 = hh // 2, (hh % 2) * 64
                        qs = qT[tile_i][row0:row0 + 64, bseq * T:(bseq + 1) * T]
                        exps = []
                        for kc in range(2):
                            ks = kT[tile_i][row0:row0 + 64,
                                            bseq * T + kc * 128: bseq * T + (kc + 1) * 128]
                            ps_s = ps_att.tile([128, 384], F32, space="PSUM", tag="att")
                            nc.tensor.matmul(ps_s[:, :T], ks, qs, start=True, stop=True)
                            e = exp_p.tile([128, T], F32, tag="expT")
                            if flags["has_mask"]:
                                nc.scalar.activation(
                                    e[:], ps_s[:, :T], AF.Exp,
                                    bias=maskb[bseq * T + kc * 128:
                                               bseq * T + (kc + 1) * 128, :],
                                    scale=0.125)
                            else:
                                nc.scalar.activation(e[:], ps_s[:, :T], AF.Exp,
                                                     bias=0.0, scale=0.125)
                            exps.append(e)
                        ps_den = ps_sm.tile([1, T], F32, space="PSUM", tag="sm1")
                        for kc in range(2):
                            nc.tensor.matmul(ps_den[:], ones_col[:], exps[kc][:],
                                             start=(kc == 0), stop=(kc == 1))
                        ps_ctx = ps_att.tile([128, 384], F32, space="PSUM", tag="att")
                        for kc in range(2):
                            vs = v_tm[bseq * 2 + kc][:, hh * 64:(hh + 1) * 64]
                            nc.tensor.matmul(ps_ctx[:64, :T], vs, exps[kc][:],
                                             start=(kc == 0), stop=(kc == 1))
                        recip = misc_p.tile([1, T], F32, tag="recip")
                        nc.vector.reciprocal(recip[:], ps_den[:])
                        ps_rb = ps_att.tile([128, 384], F32, space="PSUM", tag="att")
                        nc.tensor.matmul(ps_rb[:64, :T], ones_row[0:1, :64], recip[:],
                                         start=True, stop=True)
                        rb_sb = misc_p.tile([64, T], F32, tag="rb_sb")
                        nc.vector.tensor_copy(rb_sb[:], ps_rb[:64, :T])
                        nc.vector.tensor_tensor(
                            ctxT[tile_i][row0:row0 + 64, bseq * T:(bseq + 1) * T],
                            ps_ctx[:64, :T], rb_sb[:], op=OP.mult)

                # ---- O-projection + residual (in-place into x) ----
                wo = w_p.tile([128, KH, H], F32, tag="wqkvo")
                nc.sync.dma_start(
                    wo[:], d["w_qkvo"][lidx, 3].rearrange("(ko p) m -> p ko m", p=128))
                for mh in range(KH):
                    ps = ps_proj.tile([128, NTOK], F32, space="PSUM", tag="proj")
                    for kh in range(KH):
                        nc.tensor.matmul(ps[:], wo[:, kh, mh * 128:(mh + 1) * 128],
                                         ctxT[kh][:],
                                         start=(kh == 0), stop=(kh == KH - 1))
                    if flags["has_bias"]:
                        nc.vector.scalar_tensor_tensor(
                            out=x[mh][:], in0=ps[:], scalar=bcols[:, 2, mh:mh + 1],
                            in1=x[mh][:], op0=OP.add, op1=OP.add)
                    else:
                        nc.vector.tensor_tensor(x[mh][:], ps[:], x[mh][:], op=OP.add)
                x = layer_norm_fm(x, lidx, 0)

                # ---- FFN (x2 := x after LN1; gelu -> gT; FFN2 + residual) ----
                gT = []
                for mf in range(KF):
                    w1s = w1_p.tile([128, KH, 128], F32, tag="w1s")
                    nc.sync.dma_start(
                        w1s[:], d["w_ffn1"][lidx, mf].rearrange("(ko p) m -> p ko m", p=128))
                    ps = ps_proj.tile([128, NTOK], F32, space="PSUM", tag="proj")
                    for kh in range(KH):
                        nc.tensor.matmul(ps[:], w1s[:, kh, :], x[kh][:],
                                         start=(kh == 0), stop=(kh == KH - 1))
                    g = g_p.tile([128, NTOK], F32, tag=f"gT_{mf}")
                    if flags["has_bias"]:
                        nc.scalar.activation(g[:], ps[:], AF.Gelu,
                                             bias=bcols[:, 3, mf:mf + 1], scale=1.0)
                    else:
                        nc.scalar.activation(g[:], ps[:], AF.Gelu, bias=0.0, scale=1.0)
                    gT.append(g)
                for mh in range(KH):
                    ps = ps_proj.tile([128, NTOK], F32, space="PSUM", tag="proj")
                    for half in range(2):
                        w2s = w2_p.tile([128, KF // 2, 128], F32, tag="w2s")
                        nc.sync.dma_start(
                            w2s[:], d["w_ffn2"][lidx, mh,
                                                half * 1536:(half + 1) * 1536]
                            .rearrange("(ko p) m -> p ko m", p=128))
                        for kf2 in range(KF // 2):
                            kf = half * (KF // 2) + kf2
                            nc.tensor.matmul(ps[:], w2s[:, kf2, :], gT[kf][:],
                                             start=(kf == 0), stop=(kf == KF - 1))
                    if flags["has_bias"]:
                        nc.vector.scalar_tensor_tensor(
                            out=x[mh][:], in0=ps[:], scalar=bcols[:, 4, mh:mh + 1],
                            in1=x[mh][:], op0=OP.add, op1=OP.add)
                    else:
                        nc.vector.tensor_tensor(x[mh][:], ps[:], x[mh][:], op=OP.add)
                x = layer_norm_fm(x, lidx, 1)

        # ===================================================================
        # feats projection + featsV rearrange + Viterbi
        # ===================================================================
        with tc.tile_pool(name="vit", bufs=1) as vit_p:
            ps_f = ps_sm.tile([11, NTOK], F32, space="PSUM", tag="sm2")
            for kh in range(KH):
                nc.tensor.matmul(ps_f[:], wout[:, kh, :], x[kh][:],
                                 start=(kh == 0), stop=(kh == KH - 1))
            featsT = vit_p.tile([NL, NTOK], F32, tag="featsT")
            if flags["has_bout"]:
                bout = misc_p.tile([NL, 1], F32, tag="bout")
                nc.sync.dma_start(bout[:], d["b_out"][:])
                nc.vector.tensor_scalar(out=featsT[:], in0=ps_f[:], scalar1=bout[:],
                                        scalar2=None, op0=OP.add)
            else:
                nc.vector.tensor_copy(featsT[:], ps_f[:])

            featsV = vit_p.tile([2, T * NL], F32, tag="featsV")
            for n in range(NL):
                src = featsT[n:n + 1, :].rearrange("p (b t) -> p b t", b=2)
                dst = featsV[:].rearrange("p (t n) -> p t n", n=NL)[:, :, n]
                nc.sync.dma_start(dst, src)

            # ---- forward DP ----
            delta = vit_p.tile([2, NL], F32, tag="delta")
            nc.sync.dma_start(delta[:], d["vit_init"][:])
            psiR = vit_p.tile([2, (T - 1) * NL], F32, tag="psiR")
            s_t = vit_p.tile([2, NL * NL], F32, tag="s_t")
            eq_t = vit_p.tile([2, NL * NL], F32, tag="eq_t")
            pr_t = vit_p.tile([2, NL * NL], F32, tag="pr_t")
            m_t = vit_p.tile([2, NL], F32, tag="m_t")

            t3 = lambda ap: ap.rearrange("b (n p) -> b n p", n=NL)
            for t in range(1, T):
                nc.vector.tensor_tensor(
                    t3(s_t[:]), t3(trans2[:]),
                    delta[:][:, None, :].broadcast_to((2, NL, NL)), op=OP.add)
                nc.vector.tensor_reduce(m_t[:], t3(s_t[:]), axis=AX.X, op=OP.max)
                nc.vector.tensor_tensor(
                    t3(eq_t[:]), t3(s_t[:]), m_t[:].to_broadcast([2, NL, NL]),
                    op=OP.is_equal)
                nc.vector.tensor_tensor(pr_t[:], eq_t[:], rev121[:], op=OP.mult)
                nc.vector.tensor_reduce(
                    psiR[:, (t - 1) * NL: t * NL], t3(pr_t[:]), axis=AX.X, op=OP.max)
                nc.vector.tensor_tensor(
                    delta[:], m_t[:], featsV[:, t * NL:(t + 1) * NL], op=OP.add)

            # ---- final argmax + score ----
            score_f = vit_p.tile([2, 1], F32, tag="score")
            nc.vector.tensor_reduce(score_f[:], delta[:], axis=AX.X, op=OP.max)
            nc.sync.dma_start(d["score_out"][:], score_f[:])
            eqf = vit_p.tile([2, NL], F32, tag="eqf")
            nc.vector.tensor_scalar(out=eqf[:], in0=delta[:], scalar1=score_f[:],
                                    scalar2=None, op0=OP.is_equal)
            prf = vit_p.tile([2, NL], F32, tag="prf")
            nc.vector.tensor_tensor(prf[:], eqf[:], rev11[:], op=OP.mult)
            selR = vit_p.tile([2, 1], F32, tag="selR")
            nc.vector.tensor_reduce(selR[:], prf[:], axis=AX.X, op=OP.max)

            path_f = vit_p.tile([2, T], F32, tag="path_f")
            onehot = vit_p.tile([2, NL], F32, tag="onehot")
            scrv = vit_p.tile([2, NL], F32, tag="scrv")
            nc.vector.tensor_scalar(out=path_f[:, T - 1:T], in0=selR[:], scalar1=-1.0,
                                    scalar2=10.0, op0=OP.mult, op1=OP.add)
            nc.vector.tensor_scalar(out=onehot[:], in0=rev11[:], scalar1=selR[:],
                                    scalar2=None, op0=OP.is_equal)
            for t in range(T - 1, 0, -1):
                nc.vector.scalar_tensor_tensor(
                    out=scrv[:], in0=psiR[:, (t - 1) * NL: t * NL], scalar=1.0,
                    in1=onehot[:], op0=OP.mult, op1=OP.mult, accum_out=selR[:])
                nc.vector.tensor_scalar(out=path_f[:, t - 1:t], in0=selR[:],
                                        scalar1=-1.0, scalar2=10.0,
                                        op0=OP.mult, op1=OP.add)
                if t > 1:
                    nc.vector.tensor_scalar(out=onehot[:], in0=rev11[:], scalar1=selR[:],
                                            scalar2=None, op0=OP.is_equal)
            path_i = vit_p.tile([2, T], I32, tag="path_i")
            nc.vector.tensor_copy(path_i[:], path_f[:])
            nc.sync.dma_start(d["path_out"][:], path_i[:])


# ---------------------------------------------------------------------------
# host side
# ---------------------------------------------------------------------------

_CACHE = {}


def _prepare(params):
    p = {k: _np(v) for k, v in params.items() if k != "layers"}
    lay = {k: _np(v) for k, v in params["layers"].items()}
    L = N_LAYERS

    flags = {
        "has_bias": any(np.any(lay[b]) for b in ("bq", "bk", "bv", "bo", "b1", "b2")),
        "has_ln_affine": bool(not np.all(lay["ln1_g"] == 1) or np.any(lay["ln1_b"])
                              or not np.all(lay["ln2_g"] == 1) or np.any(lay["ln2_b"])),
        "has_emb_affine": bool(not np.all(p["emb_ln_g"] == 1) or np.any(p["emb_ln_b"])),
        "has_bout": bool(np.any(p["out_b"])),
    }

    w_qkvo = np.stack([lay["Wq"][:L], lay["Wk"][:L], lay["Wv"][:L], lay["Wo"][:L]],
                      axis=1)
    w_ffn1 = np.ascontiguousarray(
        lay["W1"][:L].reshape(L, H, KF, 128).transpose(0, 2, 1, 3))
    w_ffn2 = np.ascontiguousarray(
        lay["W2"][:L].reshape(L, FF, KH, 128).transpose(0, 2, 1, 3))
    biases = np.zeros((L, 6, FF), np.float32)
    biases[:, 0, :H] = lay["bq"][:L]
    biases[:, 1, :H] = lay["bk"][:L]
    biases[:, 2, :H] = lay["bo"][:L]
    biases[:, 3, :] = lay["b1"][:L]
    biases[:, 4, :H] = lay["b2"][:L]
    biases[:, 5, :H] = lay["bv"][:L]
    ln_gb = np.stack([
        np.stack([lay["ln1_g"][:L], lay["ln1_b"][:L]], axis=1),
        np.stack([lay["ln2_g"][:L], lay["ln2_b"][:L]], axis=1)], axis=1)

    trans = p["transitions"].astype(np.float32)
    trans2 = np.broadcast_to(trans.reshape(1, NL * NL), (2, NL * NL))
    rev = (10.0 - np.arange(NL)).astype(np.float32)
    rev121 = np.broadcast_to(np.tile(rev, NL).reshape(1, NL * NL), (2, NL * NL))
    rev11 = np.broadcast_to(rev.reshape(1, NL), (2, NL))
    vinit = np.full((2, NL), NEG, np.float32)
    vinit[:, START_ID] = 0.0

    shared = {
        "word_emb": p["word_emb"],
        "w_qkvo": w_qkvo, "w_ffn1": w_ffn1, "w_ffn2": w_ffn2,
        "biases": biases, "ln_gb": ln_gb,
        "w_out": p["out_W"],
        "b_out": p["out_b"].reshape(NL, 1),
        "emb_ln": np.stack([p["emb_ln_g"], p["emb_ln_b"]]),
        "trans2": trans2, "rev121": rev121, "rev11": rev11,
        "vit_init": vinit,
        "ident": np.eye(128, dtype=np.float32),
        "ones_col": np.ones((128, 1), np.float32),
        "ones_row": np.ones((1, 128), np.float32),
    }
    shared = {k: np.ascontiguousarray(v.astype(np.float32, copy=False))
              for k, v in shared.items()}
    return shared, flags, p


def kernel(input_ids, segment_ids, input_mask, params):
    input_ids = _np(input_ids)
    segment_ids = _np(segment_ids)
    input_mask = _np(input_mask)
    shared, flags, p = _prepare(params)
    flags["has_mask"] = bool(np.any(input_mask != 1))

    key = (N_LAYERS, tuple(sorted(flags.items())))
    if key not in _CACHE:
        _CACHE[key] = build_module(flags)
    nc = _CACHE[key]

    pos_type = p["pos_emb"][:T][None, :, :] + p["type_emb"][segment_ids]  # [B,T,H]
    mask_bias = ((1.0 - input_mask.astype(np.float32)) * NEG)             # [B,T]

    in_maps = []
    for c in range(NCORES):
        sl = slice(c * SEQ_PER_CORE, (c + 1) * SEQ_PER_CORE)
        m = dict(shared)
        m["ids"] = np.ascontiguousarray(input_ids[sl].reshape(NTOK, 1).astype(np.int32))
        m["pt_emb"] = np.ascontiguousarray(
            pos_type[sl].reshape(NTOK, H).astype(np.float32))
        m["mask_bias"] = np.ascontiguousarray(
            mask_bias[sl].reshape(NTOK, 1).astype(np.float32))
        in_maps.append(m)

    res = run_bass_kernel_spmd(
        nc, in_maps, core_ids=list(range(NCORES)),
        trace=bool(int(os.environ.get("KERNEL_TRACE", "0"))))
    score = np.concatenate([r["score_out"].reshape(SEQ_PER_CORE) for r in res.results])
    path = np.concatenate([r["path_out"] for r in res.results], axis=0)
    kernel.last_result = res
    return score.astype(np.float32), path.astype(np.int32)
